# revision 16
# baseline (speedup 1.0000x reference)
"""Trainium2 Bass kernel: DGCNN Zernike-monomial interwiner (nn_DGCNN_8839042695322).

Computes, per point p=(x,y,z):
  out[.., 16, 4] = concat_l( einsum(zernike_monoms(p)[l], Wl) ) for l=0..3
All weights fold host-side into per-channel scalar immediates (program cached
per weight set).

Memory-bound; correctness gate rel_err < 2e-2. Design notes (all measured
on hardware):
  - All compute ops run ~1.19x the isolated-probe cost once the 16 DMA
    engines stream concurrently (SBUF contention tax). ~11.4us of fixed
    preamble+input latency precedes the first compute op; ~4.5us of sem
    postamble trails the last DMA. Optimization target is therefore
    max(DVE, ACT, DMA) between those walls.
  - Single T=1024 iteration; DVE+ACT only (GPSIMD poisons concurrent DVE
    3.6x; Pool TS is 14ns/elem).
  - fp8 rows (40): l2 non-anchor units (15), l3 non-anchor units (21),
    l0 (4). f16 rows (24): l1 (12), l3 anchor (7), l2 anchor (5).
    Measured rel err 9.2e-3 vs 2e-2 gate (2.2x margin). l1 u0/u1 stay f16:
    their fp8 TT runs at 1x and costs more DVE than the DMA bytes saved.
  - Anchor tricks: l3-anchor rows ARE the bl3 bases (w3[anchor] folded into
    base constants); l2-anchor rows ARE the b2 bases (w2[anchor2] folded).
    Other units are single scaled copies (ACT any-dtype 0.83ns/el; DVE
    TS-fp8 keeps 2x mode at 0.54ns/el).
  - Pair-merged TTs: [b2_3,b2_1] via pz broadcast, [m15,m9], [m10,m14],
    [m13,m11] via row ordering chosen so operands are adjacent/broadcast.
  - Input loads split px/py (sync queue) + pz (scalar queue) so the two
    DMA queues overlap their ~7us cold-start latency; the DVE stream ends
    with a single 1-row fp8 chunk to minimize the final DMA drain.
  - l3 bases built in two halves (4 rows then 3) with the fp8 unit copies
    split 4T/3T keyed to each half, so ACT's copy pipeline and the DMA
    drain run concurrently with DVE's l1/sp mid-game instead of stacking
    at the end.
  - Every sync-queue dma_start costs DMA engine E79 ~0.2us of descriptor
    fetch on top of its data share (measured: E79 35.7us busy vs siblings'
    30.6 for equal bytes), so adjacent row-chunks whose DMA lands in the
    backlogged mid-game are issued as one dma_start.
  - Equilibrium (clean run): DVE ~32.5us, ACT ~34us, DMA ~37us busy, all
    gap-free; ~11us fixed preamble+input latency precedes compute and
    ~2.9us sem postamble trails the last DMA. All three resources sit at
    ~35us, so further gains require cutting bytes or total ops, not
    reordering.

Sharding: pure data parallel over batch across 8 NeuronCores.
"""

import numpy as np

import concourse.bacc as bacc
import concourse.tile as tile
from concourse import mybir
from concourse.bass_utils import run_bass_kernel_spmd

B, N, M_CORES = 32, 32768, 8
PTS_PER_CORE = B * N // M_CORES  # 131072
P = 128
T = PTS_PER_CORE // P            # 1024

C0 = 0.28209479177387814
C1 = 0.4886025119029199
C2_XY = 1.0925484305920792
C2_0 = 0.31539156525252005
C2_2 = 0.5462742152960396
C3_3 = 0.5900435899266435
C3_2 = 2.890611442640554
C3_1 = 0.4570457994644658
C3_0 = 0.3731763325901154
C3_P2 = 1.445305721320277

# b2 tile row position -> mm (l2 basis index): [xz, yz, xy, x2-y2, z-term]
B2_MM2POS = [2, 1, 4, 0, 3]
# l3 base row position -> k (m-9): [m15, m9, m10, m14, m13, m11, m12]
L3_K2POS = [1, 2, 5, 6, 4, 3, 0]

# f16 rows: 0:6 l1 u2/u3 (2 per m), 6:13 l3 anchor bases, 13:18 l2 anchor,
# 18:24 l1 u0/u1 (2 per m)
N16 = 24
# f8 rows: 0:15 l2 non-anchor units, 15:36 l3 non-anchor units (3x7),
# 36:40 l0
N8 = 40

_cache: dict = {}


def _host_constants(W0, b0, W1, W2, W3):
    f64 = np.float64
    A0 = (C0 * W0[0].astype(f64) + b0.astype(f64)).astype(np.float32)
    B0 = (C0 * W0[1].astype(f64)).astype(np.float32)
    AA1 = (C1 * W1[0].astype(f64)).astype(np.float32)
    BB1 = (C1 * W1[1].astype(f64)).astype(np.float32)
    w2u = W2[0].astype(f64).astype(np.float32)
    w3u = W3[0].astype(f64).astype(np.float32)
    anc3 = int(np.argmax(np.abs(w3u)))
    anc2 = int(np.argmax(np.abs(w2u)))
    return dict(A0=A0, B0=B0, AA1=AA1, BB1=BB1, w2u=w2u, w3u=w3u,
                anc3=anc3, anc2=anc2)


def _build_program(consts):
    f16 = mybir.dt.float16
    f8 = mybir.dt.float8e4
    F = mybir.ActivationFunctionType
    ALU = mybir.AluOpType
    A0, B0 = consts["A0"], consts["B0"]
    AA1, BB1 = consts["AA1"], consts["BB1"]
    w2u, w3u = consts["w2u"], consts["w3u"]
    anc3, anc2 = consts["anc3"], consts["anc2"]
    w3 = float(w3u[anc3])
    w2 = float(w2u[anc2])
    oth3 = [u for u in range(4) if u != anc3]
    oth2 = [u for u in range(4) if u != anc2]

    nc = bacc.Bacc(
        "TRN2", target_bir_lowering=False, debug=False, num_devices=M_CORES
    )
    xin = nc.dram_tensor("xin", [P, 3 * T], f16, kind="ExternalInput").ap()
    y16 = nc.dram_tensor("y16", [P, N16 * T], f16, kind="ExternalOutput").ap()
    y8 = nc.dram_tensor("y8", [P, N8 * T], f8, kind="ExternalOutput").ap()

    with tile.TileContext(nc) as tc:
        with (
            tc.tile_pool(name="xp", bufs=1) as xp,
            tc.tile_pool(name="wk", bufs=1) as wk,
            tc.tile_pool(name="op", bufs=1) as op_,
        ):
            xt = xp.tile([P, 3 * T], f16, name="xt")
            nc.sync.dma_start(out=xt[:, 0:T], in_=xin[:, 0:T])
            nc.sync.dma_start(out=xt[:, T : 2 * T], in_=xin[:, T : 2 * T])
            nc.scalar.dma_start(out=xt[:, 2 * T : 3 * T], in_=xin[:, 2 * T :])
            px, py, pz = xt[:, 0:T], xt[:, T : 2 * T], xt[:, 2 * T : 3 * T]
            xt2 = xt[:, 0 : 2 * T].rearrange("p (a b) -> p a b", a=2)

            def pl(tag, k=1):
                return wk.tile([P, k * T], f16, name=tag)

            x2, y2, z2 = pl("x2"), pl("y2"), pl("z2")
            n2a, n2, x2my2, cn2_0 = pl("n2a"), pl("n2"), pl("x2my2"), pl("cn")
            cxy = pl("cxy", 2)
            cpq = pl("cpq", 2)
            cz2 = pl("cz2", 2)   # [czA, czB]
            sp = pl("sp", 4)
            ab3 = pl("ab3", 2)   # [b3, a3]
            cnA, u5nC, d3, czC = pl("cnA"), pl("u5nC"), pl("d3"), pl("czC")
            o16 = op_.tile([P, N16 * T], f16, name="o16")
            o8 = op_.tile([P, N8 * T], f8, name="o8")

            def r16(r, k=1):
                return o16[:, r * T : (r + k) * T]

            def r8(r, k=1):
                return o8[:, r * T : (r + k) * T]

            def odma16(r0, r1):
                nc.sync.dma_start(
                    out=y16[:, r0 * T : r1 * T], in_=r16(r0, r1 - r0))

            def odma8(r0, r1):
                nc.sync.dma_start(
                    out=y8[:, r0 * T : r1 * T], in_=r8(r0, r1 - r0))

            STT = nc.vector.scalar_tensor_tensor
            TT_MUL = nc.vector.tensor_mul
            TT_ADD = nc.vector.tensor_add
            TT_SUB = nc.vector.tensor_sub

            def TS(out, in_, s1, s2=None, dst=None):
                if s2 is None:
                    nc.vector.tensor_scalar(
                        out=out, in0=in_, scalar1=float(s1), scalar2=None,
                        op0=ALU.mult)
                else:
                    nc.vector.tensor_scalar(
                        out=out, in0=in_, scalar1=float(s1), scalar2=float(s2),
                        op0=ALU.mult, op1=ALU.add)

            def bc2(v):
                return v.unsqueeze(1).broadcast_to([P, 2, T])

            # === ACT: squares (overlap DVE's product chain) ===
            nc.scalar.activation(x2, px, F.Square)
            nc.scalar.activation(y2, py, F.Square)
            nc.scalar.activation(z2, pz, F.Square)

            # === DVE: l2-anchor product rows (y16 rows 13,14,15) ===
            TS(cxy[:, 0:T], px, C2_XY * w2)             # c*px (gates on px)
            TS(cxy[:, T : 2 * T], py, C2_XY * w2)       # c*py
            cxy2 = cxy.rearrange("p (a b) -> p a b", a=2)
            TT_MUL(r16(13, 2).rearrange("p (a b) -> p a b", a=2),
                   cxy2, bc2(pz))                       # [b2_3, b2_1]
            TT_MUL(r16(15), cxy[:, 0:T], py)            # b2_0
            odma16(13, 16)

            # === ACT: l2 non-anchor early copies (rows 13:16 -> f8) ===
            for j, u in enumerate(oth2):
                nc.scalar.activation(
                    r8(5 * j, 3), r16(13, 3), F.Copy,
                    scale=float(w2u[u] / w2))
                odma8(5 * j, 5 * j + 3)

            # === DVE: n2 chain + b2 rows 16,17 ===
            TT_SUB(x2my2, x2, y2)
            TT_ADD(n2a, x2, y2)
            TT_ADD(n2, n2a, z2)
            TS(r16(16), x2my2, C2_2 * w2)               # b2_4
            TS(cn2_0, n2, C2_0 * w2)
            STT(r16(17), z2, 3.0 * C2_0 * w2, cn2_0,
                op0=ALU.mult, op1=ALU.subtract)         # b2_2
            odma16(16, 18)

            # === ACT: l2 non-anchor late copies (rows 16:18 -> f8) ===
            for j, u in enumerate(oth2):
                nc.scalar.activation(
                    r8(5 * j + 3, 2), r16(16, 2), F.Copy,
                    scale=float(w2u[u] / w2))
                odma8(5 * j + 3, 5 * j + 5)

            # === DVE: l3 bases half 1 (rows 6:10 = m15,m9,m10,m14) ===
            STT(ab3[:, 0:T], x2my2, 2.0, n2a,
                op0=ALU.mult, op1=ALU.subtract)         # b3 = x2-3y2
            STT(ab3[:, T : 2 * T], x2my2, 2.0, n2a,
                op0=ALU.mult, op1=ALU.add)              # a3 = 3x2-y2
            TS(cpq, xt[:, 0 : 2 * T], C3_3 * w3)        # [c*px, c*py]
            TT_MUL(r16(6, 2).rearrange("p (a b) -> p a b", a=2),
                   cpq.rearrange("p (a b) -> p a b", a=2),
                   ab3.rearrange("p (a b) -> p a b", a=2))  # [m15, m9]
            TS(cz2[:, 0:T], pz, C3_2 / C2_XY / w2 * w3)     # czA (x b2_0)
            TS(cz2[:, T : 2 * T], pz, C3_P2 / C2_2 / w2 * w3)  # czB (x b2_4)
            TT_MUL(r16(8, 2).rearrange("p (a b) -> p a b", a=2),
                   cz2.rearrange("p (a b) -> p a b", a=2),
                   r16(15, 2).rearrange("p (a b) -> p a b", a=2))  # [m10,m14]
            odma16(6, 10)

            # === ACT: l3 u1/u3 first 4T pieces (rows 6:10) ===
            s_oth = [float(w3u[u] / w3) for u in oth3]
            nc.scalar.activation(r8(15, 4), r16(6, 4), F.Copy, scale=s_oth[0])
            odma8(15, 19)
            nc.scalar.activation(r8(29, 4), r16(6, 4), F.Copy, scale=s_oth[2])

            # === DVE: l0 u0..u2, sp, l1 f16 (rows 0:6) ===
            for u in (0, 1, 2):
                nc.vector.tensor_scalar(
                    out=r8(36 + u), in0=n2, scalar1=float(B0[u]),
                    scalar2=float(A0[u]), op0=ALU.mult, op1=ALU.add)
            odma8(36, 39)
            for u in range(4):
                nc.vector.tensor_scalar(
                    out=sp[:, u * T : (u + 1) * T], in0=n2,
                    scalar1=float(BB1[u]), scalar2=float(AA1[u]),
                    op0=ALU.mult, op1=ALU.add)
            spA = sp[:, 2 * T : 4 * T].rearrange("p (a b) -> p a b", a=2)
            spB = sp[:, 0 : 2 * T].rearrange("p (a b) -> p a b", a=2)
            TT_MUL(r16(0, 2).rearrange("p (a b) -> p a b", a=2),
                   spA, bc2(py))
            TT_MUL(r16(2, 2).rearrange("p (a b) -> p a b", a=2),
                   spA, bc2(pz))
            TT_MUL(r16(4, 2).rearrange("p (a b) -> p a b", a=2),
                   spA, bc2(px))
            odma16(0, 6)

            # === DVE: l3 bases half 2 (rows 10:13 = m13,m11,m12) ===
            TS(cnA, n2, C3_1 * w3)
            STT(u5nC, z2, 5.0 * C3_1 * w3, cnA,
                op0=ALU.mult, op1=ALU.subtract)         # c31*w3*(5z2-n2)
            TT_MUL(r16(10, 2).rearrange("p (a b) -> p a b", a=2),
                   xt2, bc2(u5nC))                      # [m13, m11]
            STT(d3, n2, 2.0 * C3_1 * w3, u5nC,
                op0=ALU.mult, op1=ALU.subtract)         # -c31*w3*(5z2-3n2)
            TS(czC, pz, -C3_0 / C3_1)
            TT_MUL(r16(12), czC, d3)                    # m12
            odma16(10, 13)

            # === ACT: l3 u1/u3/u2 3T pieces (rows 10:13) ===
            nc.scalar.activation(r8(19, 3), r16(10, 3), F.Copy, scale=s_oth[0])
            odma8(19, 22)
            nc.scalar.activation(r8(33, 3), r16(10, 3), F.Copy, scale=s_oth[2])
            odma8(29, 36)
            nc.scalar.activation(r8(26, 3), r16(10, 3), F.Copy, scale=s_oth[1])

            # === DVE: l1 u0/u1 (f16, keeps 2x TT mode) ===
            TT_MUL(r16(18, 2).rearrange("p (a b) -> p a b", a=2), spB, bc2(py))
            TT_MUL(r16(20, 2).rearrange("p (a b) -> p a b", a=2), spB, bc2(pz))
            TT_MUL(r16(22, 2).rearrange("p (a b) -> p a b", a=2), spB, bc2(px))
            odma16(18, 24)

            # === DVE: l3 u2 first 4T piece + last l0 row (tiny tail) ===
            nc.vector.tensor_scalar(
                out=r8(22, 4), in0=r16(6, 4), scalar1=s_oth[1], scalar2=None,
                op0=ALU.mult)
            odma8(22, 29)
            nc.vector.tensor_scalar(
                out=r8(39), in0=n2, scalar1=float(B0[3]),
                scalar2=float(A0[3]), op0=ALU.mult, op1=ALU.add)
            odma8(39, 40)

    nc.compile()
    return nc


def _get_program(consts):
    key = tuple(
        consts[k].tobytes() for k in ("A0", "B0", "AA1", "BB1", "w2u", "w3u")
    ) + ("v3", consts["anc3"], consts["anc2"])
    if _cache.get(key) is None:
        _cache[key] = _build_program(consts)
    return _cache[key]


def _prep_inputs(x):
    xs = np.asarray(x, dtype=np.float32).reshape(M_CORES, P, T, 3)
    xs = np.ascontiguousarray(xs.transpose(0, 1, 3, 2)).astype(np.float16)
    return xs.reshape(M_CORES, P, 3 * T)


def _make_index_maps(anc3, anc2):
    oth3 = [u for u in range(4) if u != anc3]
    oth2 = [u for u in range(4) if u != anc2]
    ch16, idx16, ch8, idx8 = [], [], [], []
    for ch in range(64):
        m, u = ch // 4, ch % 4
        if m == 0:
            ch8.append(ch); idx8.append(36 + u)
        elif 1 <= m <= 3:
            if u >= 2:
                ch16.append(ch); idx16.append(2 * (m - 1) + (u - 2))
            else:
                ch16.append(ch); idx16.append(18 + 2 * (m - 1) + u)
        elif 4 <= m <= 8:
            mm = m - 4
            if u == anc2:
                ch16.append(ch); idx16.append(13 + B2_MM2POS[mm])
            else:
                j = oth2.index(u)
                ch8.append(ch); idx8.append(5 * j + B2_MM2POS[mm])
        else:
            k = m - 9
            if u == anc3:
                ch16.append(ch); idx16.append(6 + L3_K2POS[k])
            else:
                j = oth3.index(u)
                ch8.append(ch); idx8.append(15 + 7 * j + L3_K2POS[k])
    return (np.array(ch16), np.array(idx16, dtype=np.int64),
            np.array(ch8), np.array(idx8, dtype=np.int64))


def _reconstruct(results, anc3, anc2):
    ch16, idx16, ch8, idx8 = _make_index_maps(anc3, anc2)
    out = np.empty((M_CORES, P, T, 64), dtype=np.float32)
    for c in range(M_CORES):
        a16 = results[c]["y16"].reshape(P, N16, T)
        a8 = results[c]["y8"].reshape(P, N8, T)
        out[c][:, :, ch16] = (
            a16[:, idx16, :].astype(np.float32).transpose(0, 2, 1))
        out[c][:, :, ch8] = (
            a8[:, idx8, :].astype(np.float32).transpose(0, 2, 1))
    return out.reshape(B, N, 16, 4)


def _run(x, W0, b0, W1, W2, W3, trace=False):
    consts = _host_constants(
        np.asarray(W0, np.float32), np.asarray(b0, np.float32),
        np.asarray(W1, np.float32), np.asarray(W2, np.float32),
        np.asarray(W3, np.float32),
    )
    nc = _get_program(consts)
    xin = _prep_inputs(x)
    in_maps = [{"xin": xin[c]} for c in range(M_CORES)]
    kwargs = {}
    if trace:
        kwargs = dict(trace=True, trace_cores=[0])
    res = run_bass_kernel_spmd(nc, in_maps, list(range(M_CORES)), **kwargs)
    out = _reconstruct(res.results, consts["anc3"], consts["anc2"])
    return out, res


def kernel(x, W0, b0, W1, W2, W3):
    out, _ = _run(x, W0, b0, W1, W2, W3)
    return out


def kernel_traced(x, W0, b0, W1, W2, W3):
    import sys
    import types

    if "antenv.axon_hooks" not in sys.modules:
        mod = types.ModuleType("antenv.axon_hooks")
        _h = [None]
        mod.set_axon_ntff_profile_hook = lambda h: _h.__setitem__(0, h)
        mod.get_axon_ntff_profile_hook = lambda: _h[0]
        sys.modules["antenv.axon_hooks"] = mod
        if "/root/.axon_site" not in sys.path:
            sys.path.insert(0, "/root/.axon_site")
        from trn_agent_boot.trn_boot import _ntff_profile_via_ctypes

        mod.set_axon_ntff_profile_hook(
            _ntff_profile_via_ctypes("/opt/axon/libaxon_pjrt.so")
        )
    import concourse.bass_utils as bu

    bu.upload_artifacts = lambda tmpdir: "local://" + tmpdir
    return _run(x, W0, b0, W1, W2, W3, trace=True)
